# revision 31
# baseline (speedup 1.0000x reference)
"""Binary Conv2d (sign-act 3x3 binary conv + RPReLU + residual) on 8 trn2 NeuronCores.

Reference computation (forward values):
  a  = sign(x + move0_bias)                       # {-1,0,+1}
  bw = scale_o * sign(conv_w), scale_o = mean |conv_w| over (I,KH,KW)
  z  = conv2d(a, bw, pad=1) + pr_bias0
  y  = where(z>=0, z, alpha*z) + pr_bias1 + x

Strategy: data-parallel over batch (16 imgs -> 2 per core). Conv as 9 tap
matmuls with fp8e4 DoubleRow (contracts both 128-channel chunks per matmul,
2 rows/cycle pipelined = 157 TF/s) accumulating in PSUM. Acts are exact
sign values in fp8 stored row-major [padded_row][ic][66] so 3x3 windows
are plain strided slices. PE stream is gapless: 288 matmuls x 216ns =
62.3us/core, the fp8 roofline for this conv.

Fast path (all biases zero, alpha<1 -- the reference's setup):
  - fp16 IO, x shipped host-padded to width 66 (zero cols) so the device
    Sign writes produce zero borders for free; y out fp16, host upcasts.
  - Startup is HWDGE-bound: one serial descriptor generator (~19ns/desc)
    and a fixed partition-block -> DMA-engine map mean a transfer lands at
    first-desc (8.1us) + gen(#descs) + 8 descs on the last engine. boot1a
    = w-oc0 + epilogue consts + pre-signed act rows 0..17 (4712B, lands
    ~12.2us); boot1b = w-oc1; boot2 = act rows 18..33. Units run
    oc-interleaved so boot rows 0..33 give 15.5us of PE work while x
    lands and the device signs rows 31..63 (one Act op per image).
  - A continuous stream of small warm-up matmuls bridges until boot1a
    lands: both the HAM array-width gate (k=4 after idle) and the p-state
    clock (1.2GHz for ~3us after idle) punish any PE gap.
  - Epilogue per 16-row unit is a 3-op DVE chain: r = (1-a)s*Relu(p),
    v = a*s*p + x, y = r + v. Keeping the PSUM-releasing ops on the DVE
    matters: the tile scheduler orders same-queue ops by its own sim and
    parks DMA-gated sign ops behind Act-engine epilogues (5.6us stall).
  - Tail: last 16 rows split into two 8-row units (same matmul count);
    those two use u = Prelu(s*p, alpha) on the idle Act engine + one fp16
    add, and the final store splits across both HWDGE rings.
General path (any nonzero bias): f32 IO, previous-generation structure.
"""

import sys
for _p in ("/opt/trn_rl_repo",):
    if _p not in sys.path:
        sys.path.append(_p)

from contextlib import ExitStack

import numpy as np
import ml_dtypes

import concourse.bass as bass
import concourse.tile as tile
from concourse import bacc, mybir
from concourse import bass_utils

N_CORES = 8
B, C, H, W = 16, 256, 64, 64
K = 3
BPC = B // N_CORES            # imgs per core
NCH = C // 128                # channel chunks (2)
SP = H * W                    # spatial 4096
PW = W + 2                    # padded width 66
PXW = H * PW                  # padded x elems per (img, ic) 4224
NTAP = K * K
RB = 8                        # out rows per matmul bank
WB = NTAP * NCH * NCH * 128   # weight bytes per partition (4608)
CSTB = NCH * 4 * 4            # epilogue-const bytes per partition (32)
ROWB = NCH * PW               # act bytes per padded row (132)
BOOT_HROWS = 34               # padded act rows 0..33 shipped by host
B1ROWS = 18                   # boot1a carries padded rows 0..17
WOCB = NTAP * NCH * 128       # weight bytes per partition per oc chunk (2304)
# boot tile layout per partition:
#   [w_oc0 | cst | acts rows 0..33 | w_oc1]
OFF_CST = WOCB                       # 2304
OFF_ACT = WOCB + CSTB                # 2336
OFF_W1 = OFF_ACT + BOOT_HROWS * ROWB  # 6824
BOOTB = OFF_W1 + WOCB                # 9128
BOOT1AB = OFF_ACT + B1ROWS * ROWB    # 4712: w_oc0 + cst + act rows 0..17
AT0_BASE = 32                 # at0 h-index 0 == padded row 32 (img 0)
AT0_ROWS = H + 2 - AT0_BASE   # 34

N_WARM = 23                   # small matmuls to keep the PE continuously
                              # busy (HAM width + p-state clock both drop
                              # after idle) until the boot transfer lands
                              # (~11.9us with the 8-way interleaved boot;
                              # landing is pinned there for 1/4/8-way
                              # splits -- sem-wait + desc-exec bound);
                              # plain (non-DR) fp8 warms measured best --
                              # DR-mode warms delayed HAM k=8 engagement
# Mid-stream epilogue is a 3-op DVE chain: putting the PSUM-releasing op on
# the Act engine (parametric relu) lets the tile scheduler order it against
# the DMA-gated sign ops on the same queue, and its DMA timing model is
# wrong enough that it parked sign-b0 behind four epilogues (5.6us PE
# stall).  Only the final two taper units use Prelu+add (Act is idle by
# then, so any order works) to shorten the tail chain.  A 2-op DVE form is
# illegal: a DVE op may read only one PSUM operand (NCC_IBVF027).

F32 = mybir.dt.float32
FP16 = mybir.dt.float16
FP8 = mybir.dt.float8e4
U8 = mybir.dt.uint8

_CACHE = {}


def _build_program(io_fp16: bool):
    nc = bacc.Bacc(
        "TRN2",
        target_bir_lowering=False,
        debug=False,
        enable_asserts=False,
        num_devices=N_CORES,
    )
    if io_fp16:
        x_d = nc.dram_tensor("x", [BPC, C, PXW], FP16, kind="ExternalInput").ap()
        y_d = nc.dram_tensor("y", [BPC, C, SP], FP16, kind="ExternalOutput").ap()
        wx1_d = nc.dram_tensor("wx1", [128, BOOT1AB], U8,
                               kind="ExternalInput").ap()
        wx1b_d = nc.dram_tensor("wx1b", [128, WOCB], U8,
                                kind="ExternalInput").ap()
        wx2_d = nc.dram_tensor("wx2", [128, OFF_W1 - BOOT1AB], U8,
                               kind="ExternalInput").ap()
        with tile.TileContext(nc) as tc:
            _kernel_fast(tc, y_d, x_d, wx1_d, wx1b_d, wx2_d)
    else:
        x_d = nc.dram_tensor("x", [BPC, C, SP], F32, kind="ExternalInput").ap()
        y_d = nc.dram_tensor("y", [BPC, C, SP], F32, kind="ExternalOutput").ap()
        w_d = nc.dram_tensor("w", [128, WB], FP8, kind="ExternalInput").ap()
        cst_d = nc.dram_tensor("cst", [C, 4], F32, kind="ExternalInput").ap()
        with tile.TileContext(nc) as tc:
            _kernel_general(tc, y_d, x_d, w_d, cst_d)
    nc.compile()
    return nc


def _kernel_fast(tc, y_d, x_d, wx1_d, wx1b_d, wx2_d):
    nc = tc.nc
    MULT = mybir.AluOpType.mult
    ADD = mybir.AluOpType.add
    MAX = mybir.AluOpType.max
    ctx = ExitStack()
    with ctx:
        const = ctx.enter_context(tc.tile_pool(name="const", bufs=1))
        xpool = ctx.enter_context(tc.tile_pool(name="x", bufs=1))
        apool = ctx.enter_context(tc.tile_pool(name="act", bufs=1))
        work = ctx.enter_context(tc.tile_pool(name="work", bufs=3))
        psum = ctx.enter_context(tc.tile_pool(name="psum", bufs=4, space="PSUM"))

        # --- tiles; act/x tiles are flat with dual (mm / sign) views so all
        # APs are unflatten+slice (no permutes of sliced APs) ---
        xt = {}    # b -> [128, NCH*PXW] fp16 padded x (sign source + residual)
        xmm = {}   # [128, ic, h, w] view
        xsv = {}   # [128, h, ic, w] view (sign source ordering)
        for b in range(BPC):
            xt[b] = xpool.tile([128, NCH * PXW], FP16, tag=f"xt{b}",
                               name=f"xt{b}")
            xmm[b] = xt[b][:].rearrange("p (i h w) -> p i h w", i=NCH, w=PW)
            xsv[b] = xt[b][:].rearrange("p (i h w) -> p h i w", i=NCH, w=PW)
        at0 = apool.tile([128, AT0_ROWS * ROWB], FP8, tag="at0", name="at0")
        at0_mm = at0[:].rearrange("p (h i w) -> p i h w", i=NCH, w=PW)
        at0_s = at0[:].rearrange("p (h i w) -> p h i w", i=NCH, w=PW)
        at1 = apool.tile([128, (H + 2) * ROWB], FP8, tag="at1", name="at1")
        at1_mm = at1[:].rearrange("p (h i w) -> p i h w", i=NCH, w=PW)
        at1_s = at1[:].rearrange("p (h i w) -> p h i w", i=NCH, w=PW)
        boot = const.tile([128, BOOTB], U8, tag="boot", name="boot")
        # per-oc weight bases: oc0 at 0, oc1 at OFF_W1
        wbase = {0: 0, 1: OFF_W1}
        cstt = boot[:, OFF_CST:OFF_CST + CSTB].bitcast(F32).rearrange(
            "p (o f) -> p o f", o=NCH)
        bact_mm = boot[:, OFF_ACT:OFF_W1].bitcast(FP8).rearrange(
            "p (h i w) -> p i h w", i=NCH, w=PW)
        warm = const.tile([128, 512], FP8, tag="warm")
        scratch = const.tile([128, 1], F32, tag="scr", name="scratch")

        # --- DMA configs: first-needed bytes lead.  Descriptor generation
        # is serial (~14.5ns/desc, one generator shared by both HWDGE
        # rings) and descriptors map to the 16 DMA engines in partition
        # blocks of 8 (engine = partition//8), so a single 128-desc
        # transfer makes the engine owning partitions 120..127 execute all
        # 8 of its descriptors after the whole generation chain.  boot1a
        # is split into 8 transfers over partitions {r, r+8, .., r+120}
        # (one descriptor per engine each, configs alternating between the
        # two rings) so every engine's work spreads across the window. ---
        bootv = boot[:, 0:BOOT1AB].rearrange("(a b) x -> a b x", b=8)
        wx1v = wx1_d.rearrange("(a b) x -> a b x", b=8)
        for r in range(8):
            eng = nc.sync if r % 2 == 0 else nc.scalar
            eng.dma_start(out=bootv[:, r, :], in_=wx1v[:, r, :])
        nc.sync.dma_start(out=boot[:, OFF_W1:BOOTB], in_=wx1b_d[:])
        nc.sync.dma_start(out=boot[:, BOOT1AB:OFF_W1], in_=wx2_d[:])
        xv = x_d.rearrange("b (i p) s -> b p i s", i=NCH)

        def dma_x(b):
            for ic in range(NCH):
                nc.sync.dma_start(out=xmm[b][:, ic, :, :], in_=xv[b, :, ic, :])

        dma_x(0)

        nc.gpsimd.memset(warm[:], 1.0)
        # preload the scalar activation table off the critical path
        nc.scalar.activation(scratch[:], warm[:, 0:1],
                             mybir.ActivationFunctionType.Sign,
                             bias=0.0, scale=1.0)

        # --- PE warm-up while startup DMAs land ---
        wps = psum.tile([128, 16, W], F32, tag="pt", name="wps")
        for _ in range(N_WARM):
            nc.tensor.matmul(wps[:, 0:4, :], warm[:, 0:128], warm[:, 0:256],
                             start=True, stop=True)

        # borders: only top/bottom rows need memsets (cols are zero because
        # x is host-padded and sign(0) == 0)
        nc.gpsimd.memset(at0_s[:, AT0_ROWS - 1:AT0_ROWS, :, :], 0.0)
        nc.gpsimd.memset(at1_s[:, 0:1, :, :], 0.0)
        nc.gpsimd.memset(at1_s[:, H + 1:H + 2, :, :], 0.0)

        # sign b0: img rows 31..63 -> at0 padded rows 32..64 (h-idx 0..32)
        nc.scalar.activation(
            at0_s[:, 0:H - 31, :, :], xsv[0][:, 31:H, :, :],
            mybir.ActivationFunctionType.Sign, bias=0.0, scale=1.0)

        def sign_b1():
            # img rows 0..63 -> at1 padded rows 1..64
            nc.scalar.activation(
                at1_s[:, 1:H + 1, :, :], xsv[1][:, :, :, :],
                mybir.ActivationFunctionType.Sign, bias=0.0, scale=1.0)

        # --- unit schedule: 16 out rows per unit (2 PSUM banks, bufs=4),
        # oc-interleaved so the first 4 units run off boot acts alone; the
        # final 16 rows split into two 8-row units (same matmul count,
        # smaller tail epilogue + the last y store on the Act HWDGE ring).
        sched = []
        for b in range(BPC):
            for r0 in range(0, H, 16):
                for oc in range(NCH):
                    last_pair = (b == BPC - 1 and oc == NCH - 1)
                    if r0 == 48 and last_pair:
                        sched.append((b, oc, 48, 8))
                        sched.append((b, oc, 56, 8))
                    else:
                        sched.append((b, oc, r0, 16))

        first_y = True
        for (bb, oc, r0u, nru) in sched:
            if bb == 0 and r0u < AT0_BASE:
                src, hbase = bact_mm, 0
            elif bb == 0:
                src, hbase = at0_mm, AT0_BASE
            else:
                src, hbase = at1_mm, 0
            pt = psum.tile([128, 16, W], F32, tag="pt")
            off = 0
            for rb0 in range(r0u, r0u + nru, RB):
                nr = min(RB, r0u + nru - rb0)
                outsl = pt[:, off:off + nr, :]
                off += nr
                for t in range(NTAP):
                    kh, kw = divmod(t, K)
                    wb0 = wbase[oc] + t * NCH * 128
                    wsl = boot[:, wb0:wb0 + NCH * 128].bitcast(FP8)
                    lhsT = wsl.rearrange("p (i m) -> p i m", i=NCH)
                    h0 = rb0 + kh - hbase
                    rhs = src[:, :, h0:h0 + nr, kw:kw + W]
                    nc.tensor.matmul(
                        outsl, lhsT, rhs,
                        start=(t == 0), stop=(t == NTAP - 1),
                        perf_mode=mybir.MatmulPerfMode.DoubleRow)
            # epilogue: y = PReLU_a(s*p) + x  (== s*PReLU_a(p) + x, s>0)
            sl = slice(0, nru)
            xsl = xmm[bb][:, oc, r0u:r0u + nru, 1:1 + W]
            yt = work.tile([128, 16, W], FP16, tag="yt", name="yt")
            tail_unit = (bb == BPC - 1 and oc == NCH - 1 and r0u >= 48)
            if tail_unit:
                # Act-engine parametric relu + one fp16 add: shortest chain
                # after the final matmul (safe here only — the Act queue is
                # empty by now, so scheduler order can't hurt)
                u = work.tile([128, 16, W], FP16, tag="u", name="u")
                nc.scalar.activation(
                    u[:, sl, :], pt[:, sl, :],
                    mybir.ActivationFunctionType.Prelu,
                    bias=0.0, scale=cstt[:, oc, 0:1],
                    alpha=cstt[:, oc, 1:2])
                nc.vector.tensor_add(out=yt[:, sl, :], in0=u[:, sl, :],
                                     in1=xsl)
            else:
                r = work.tile([128, 16, W], FP16, tag="u", name="r")
                nc.vector.tensor_scalar(
                    out=r[:, sl, :], in0=pt[:, sl, :],
                    scalar1=cstt[:, oc, 2:3], scalar2=0.0,
                    op0=MULT, op1=MAX)
                v = work.tile([128, 16, W], FP16, tag="v", name="v")
                nc.vector.scalar_tensor_tensor(
                    out=v[:, sl, :], in0=pt[:, sl, :],
                    scalar=cstt[:, oc, 3:4], in1=xsl,
                    op0=MULT, op1=ADD)
                nc.vector.tensor_add(out=yt[:, sl, :], in0=r[:, sl, :],
                                     in1=v[:, sl, :])
            is_last = (bb == BPC - 1 and oc == NCH - 1 and r0u == 56)
            ysl = y_d[bb, oc * 128:(oc + 1) * 128, r0u * W:(r0u + nru) * W]
            if is_last:
                # split the final store across both HWDGE rings by
                # partition halves so the two 64-desc generations overlap
                nc.sync.dma_start(out=ysl[0:64], in_=yt[0:64, sl, :])
                nc.scalar.dma_start(out=ysl[64:128], in_=yt[64:128, sl, :])
            else:
                nc.sync.dma_start(out=ysl, in_=yt[:, sl, :])
            if first_y:
                first_y = False
                # img 1 loads config after the first y write enters the SP
                # queue so they stay out of the startup descriptor window
                dma_x(1)
            if (bb, oc, r0u) == (0, 1, 16):
                # sign b1 emitted mid-stream: late enough that it queues
                # behind a few epilogue Act ops (so it never blocks early
                # PSUM releases), early enough to finish long before img 1
                sign_b1()


def _kernel_general(tc, y_d, x_d, w_d, cst_d):
    """f32 general path (nonzero biases): previous-generation structure."""
    nc = tc.nc
    PHR = 72
    CST = PHR * PW
    UBE = 1024
    ctx = ExitStack()
    with ctx:
        const = ctx.enter_context(tc.tile_pool(name="const", bufs=1))
        xpool = ctx.enter_context(tc.tile_pool(name="x", bufs=1))
        apool = ctx.enter_context(tc.tile_pool(name="act", bufs=1))
        work = ctx.enter_context(tc.tile_pool(name="work", bufs=3))
        psum = ctx.enter_context(tc.tile_pool(name="psum", bufs=4, space="PSUM"))

        xv = x_d.rearrange("b (i p) s -> b p i s", i=NCH)
        xt = {}
        at = {}
        for b in range(BPC):
            xt[b] = xpool.tile([128, NCH, SP], F32, tag=f"xt{b}",
                               name=f"xt{b}")
            at[b] = apool.tile([128, NCH * CST], FP8, tag=f"at{b}",
                               name=f"at{b}")
        warm = const.tile([128, 512], FP8, tag="warm")
        nc.gpsimd.memset(warm[:], 1.0)
        scratch = const.tile([128, 1], F32, tag="scr", name="scratch")

        wtt = const.tile([128, WB], FP8, tag="wt")
        wt = wtt[:]
        cstv = const.tile([128, NCH, 4], F32, tag="cst", name="cstt")
        cstt = cstv[:]
        cv = cst_d.rearrange("(i p) f -> p i f", i=NCH)
        xgrps = {b: [(0, 10), (10, 22), (22, 34), (34, 48), (48, 64)]
                 for b in range(BPC)}

        def dma_x(b):
            for (r0, r1) in xgrps[b]:
                for ic in range(NCH):
                    nc.sync.dma_start(out=xt[b][:, ic, r0 * W:r1 * W],
                                      in_=xv[b, :, ic, r0 * W:r1 * W])

        nc.sync.dma_start(out=wtt[:], in_=w_d[:])
        nc.sync.dma_start(out=cstv[:], in_=cv[:])
        for b in range(BPC):
            dma_x(b)

        nc.scalar.activation(scratch[:], warm[:, 0:1],
                             mybir.ActivationFunctionType.Sign,
                             bias=0.0, scale=1.0)

        wps = psum.tile([128, UBE], F32, tag="pt", name="wps")
        for _ in range(10):
            nc.tensor.matmul(wps[:, 0:512], warm[:, 0:128], warm[:],
                             start=True, stop=True)

        for b in range(BPC):
            a4 = at[b][:].rearrange("p (i h w) -> p i h w", i=NCH, w=PW)
            nc.gpsimd.memset(a4[:, :, 0:1, :], 0.0)
            nc.gpsimd.memset(a4[:, :, H + 1:H + 2, :], 0.0)
            nc.gpsimd.memset(a4[:, :, 1:H + 1, 0:1], 0.0)
            nc.gpsimd.memset(a4[:, :, 1:H + 1, PW - 1:PW], 0.0)

        def signs(b):
            a4 = at[b][:].rearrange("p (i h w) -> p i h w", i=NCH, w=PW)
            for (r0, r1) in xgrps[b]:
                for ic in range(NCH):
                    x3 = xt[b][:, ic, r0 * W:r1 * W].rearrange(
                        "p (h w) -> p h w", w=W)
                    nc.scalar.activation(
                        a4[:, ic, 1 + r0:1 + r1, 1:1 + W], x3,
                        mybir.ActivationFunctionType.Sign,
                        bias=cstt[:, ic, 3:4], scale=1.0)

        for b in range(BPC):
            signs(b)
            a4 = at[b][:].rearrange("p (i h w) -> p i h w", i=NCH, w=PW)
            for oc in range(NCH):
                if b == BPC - 1 and oc == NCH - 1:
                    units = [(0, 16), (16, 16), (32, 16), (48, 8),
                             (56, 4), (60, 2), (62, 2)]
                else:
                    units = [(0, 16), (16, 16), (32, 16), (48, 16)]
                for (r0u, nru) in units:
                    ube = nru * W
                    pt = psum.tile([128, UBE], F32, tag="pt")
                    off = 0
                    for rb0 in range(r0u, r0u + nru, RB):
                        nr = min(RB, r0u + nru - rb0)
                        out_half = pt[:, off:off + nr * W]
                        off += nr * W
                        for kh in range(K):
                            for kw in range(K):
                                t = kh * K + kw
                                wsl = wt[:, (t * NCH + oc) * NCH
                                         * 128:(t * NCH + oc + 1) * NCH * 128]
                                lhsT = wsl.rearrange("p (i m) -> p i m", i=NCH)
                                rhs = a4[:, :, rb0 + kh:rb0 + kh + nr,
                                         kw:kw + W]
                                nc.tensor.matmul(
                                    out_half, lhsT, rhs,
                                    start=(t == 0), stop=(t == NTAP - 1),
                                    perf_mode=mybir.MatmulPerfMode.DoubleRow)
                    base = r0u * W
                    sl = slice(0, ube)
                    xsl = xt[b][:, oc, base:base + ube]
                    r = work.tile([128, UBE], F32, tag="r", name="r")
                    nc.scalar.activation(
                        r[:, sl], pt[:, sl],
                        mybir.ActivationFunctionType.Relu,
                        bias=cstt[:, oc, 1:2], scale=cstt[:, oc, 0:1])
                    v = work.tile([128, UBE], F32, tag="v", name="v")
                    nc.vector.scalar_tensor_tensor(
                        out=v[:, sl], in0=pt[:, sl], scalar=cstt[:, oc, 2:3],
                        in1=xsl, op0=mybir.AluOpType.mult,
                        op1=mybir.AluOpType.add)
                    yt = work.tile([128, UBE], F32, tag="yt", name="yt")
                    nc.vector.tensor_add(out=yt[:, sl], in0=r[:, sl],
                                         in1=v[:, sl])
                    nc.sync.dma_start(
                        out=y_d[b, oc * 128:(oc + 1) * 128,
                                base:base + ube],
                        in_=yt[:, sl])


def _pack_inputs(x, move0_bias, conv_w, pr_bias0, prelu_alpha, pr_bias1):
    """Host-side prep: weight binarization + epilogue constant folding."""
    f32 = np.float32
    w = conv_w.astype(f32)
    scale = np.abs(w).mean(axis=(1, 2, 3)).astype(f32)          # (O,)
    ws = np.sign(w)

    alpha = prelu_alpha.astype(f32).reshape(C)
    b0 = pr_bias0.astype(f32).reshape(C)
    b1 = pr_bias1.astype(f32).reshape(C)
    mb = move0_bias.astype(f32).reshape(C)

    io_fp16 = bool(np.all(mb == 0.0) and np.all(b0 == 0.0)
                   and np.all(b1 == 0.0) and np.all(alpha < 1.0))
    x = x.astype(f32)
    in_maps = []
    if io_fp16:
        # weights laid out [p][oc][tap][ic][m] so per-(oc,tap) lhsT slices
        # are contiguous
        wsr = ws.reshape(NCH, 128, NCH, 128, NTAP)              # (oc,m,ic,p,t)
        lhsT = wsr.transpose(3, 0, 4, 2, 1)                     # (p,oc,t,ic,m)
        w_u8 = np.ascontiguousarray(lhsT.astype(
            ml_dtypes.float8_e4m3)).reshape(128, NCH, WOCB).view(np.uint8)
        cst = np.stack([scale, alpha, (1 - alpha) * scale, alpha * scale],
                       axis=1).astype(f32)                      # (C, 4)
        cst_u8 = np.ascontiguousarray(
            cst.reshape(NCH, 128, 4).transpose(1, 0, 2).reshape(128, CSTB // 4)
        ).view(np.uint8)                                        # [128, 32]
        xp = np.zeros((B, C, H, PW), np.float16)
        xp[..., 1:1 + W] = x.astype(np.float16)
        for i in range(N_CORES):
            xc = np.ascontiguousarray(
                xp[i * BPC:(i + 1) * BPC].reshape(BPC, C, PXW))
            # pre-signed padded acts for img 0, padded rows 0..33, row-major
            # [p][h][ic][w]; sign of fp16 x matches the device Sign exactly
            # and the padded zero cols sign to zero
            a = np.zeros((128, BOOT_HROWS, NCH, PW), f32)
            xs = xp[i * BPC, :, :BOOT_HROWS - 1, :].astype(f32).reshape(
                NCH, 128, BOOT_HROWS - 1, PW)
            a[:, 1:, :, :] = np.sign(xs).transpose(1, 2, 0, 3)
            a8 = a.astype(ml_dtypes.float8_e4m3).reshape(
                128, BOOT_HROWS * ROWB).view(np.uint8)
            wx1 = np.ascontiguousarray(np.concatenate(
                [w_u8[:, 0], cst_u8, a8[:, :B1ROWS * ROWB]], axis=1))
            in_maps.append({"wx1": wx1,
                            "wx1b": np.ascontiguousarray(w_u8[:, 1]),
                            "wx2": np.ascontiguousarray(a8[:, B1ROWS * ROWB:]),
                            "x": xc})
    else:
        # general path: weights [p][tap][oc][ic][m], sign bias folded
        wsr = ws.reshape(NCH, 128, NCH, 128, NTAP)              # (oc,m,ic,p,t)
        lhsT = wsr.transpose(3, 4, 0, 2, 1)                     # (p,t,oc,ic,m)
        lhsT = np.ascontiguousarray(lhsT.astype(
            ml_dtypes.float8_e4m3)).reshape(128, WB)
        c_fold = alpha * b0 + b1
        cst = np.stack([(1 - alpha) * scale, (1 - alpha) * b0,
                        alpha * scale, mb - c_fold], axis=1).astype(f32)
        xh = x.reshape(B, C, SP) + c_fold.reshape(1, C, 1)
        for i in range(N_CORES):
            in_maps.append({
                "w": lhsT, "cst": cst,
                "x": np.ascontiguousarray(xh[i * BPC:(i + 1) * BPC])})
    return in_maps, io_fp16


def kernel(x, move0_bias, conv_w, pr_bias0, prelu_alpha, pr_bias1):
    in_maps, io_fp16 = _pack_inputs(
        np.asarray(x), np.asarray(move0_bias), np.asarray(conv_w),
        np.asarray(pr_bias0), np.asarray(prelu_alpha), np.asarray(pr_bias1))
    key = ("nc", io_fp16)
    if key not in _CACHE:
        _CACHE[key] = _build_program(io_fp16)
    nc = _CACHE[key]
    res = bass_utils.run_bass_kernel_spmd(nc, in_maps,
                                          core_ids=list(range(N_CORES)))
    _CACHE["last_results"] = res
    out = np.concatenate([res.results[i]["y"] for i in range(N_CORES)], axis=0)
    return out.astype(np.float32).reshape(B, C, H, W)
